# revision 17
# baseline (speedup 1.0000x reference)
"""Segment-mean (graph pooling) kernel for Trainium2, 8 NeuronCores.

reference semantics:
    sums   = segment_sum(node_h, node_batch, num_segments=G)
    counts = segment_sum(ones(N), node_batch, G)
    out    = sums / max(counts, 1)[:, None]

node_batch is sorted, so segments are contiguous row runs. Sharding:
core c owns segments [128c, 128(c+1)) and streams the node rows that
cover them. Per 128-row tile the DVE builds a one-hot selector
(iota == local_seg_id) and the PE accumulates onehot.T @ tile into a
single PSUM tile [128 segs, 256] (fp32 hi/lo bf16 split in the free
dim, exact to ~2^-18). Epilogue adds the halves and scales by
1/max(count,1).
"""

import os

import numpy as np
import ml_dtypes

BF16 = ml_dtypes.bfloat16
P = 128  # partitions / nodes per tile / segments per core
D = 128  # feature dim
G = 1024  # num segments
N_CORES = 8
SLAB = 16  # node-tiles per DMA slab (1 MiB per slab)
SENTINEL = 200.0  # local seg id outside [0, 128) -> all-zero one-hot column

_prog_cache: dict[int, object] = {}
LAST_RESULT = None  # BassKernelResults of the most recent device run


def _np_fallback(node_h, node_batch, num_graphs):
    node_h = np.asarray(node_h, dtype=np.float32)
    nb = np.asarray(node_batch).astype(np.int64)
    ng = int(num_graphs)
    sums = np.zeros((ng, node_h.shape[1]), dtype=np.float32)
    np.add.at(sums, nb, node_h)
    counts = np.bincount(nb, minlength=ng).astype(np.float32)
    return sums / np.maximum(counts, 1.0)[:, None]


def _build_program(T: int):
    import concourse.bacc as bacc
    import concourse.mybir as mybir
    import concourse.tile as tile
    from concourse.tile import add_dep_helper

    OH_BUFS = 4

    bf16 = mybir.dt.bfloat16
    f32 = mybir.dt.float32

    nc = bacc.Bacc(None)
    h_in = nc.dram_tensor("h", [P, T * 2 * D], bf16, kind="ExternalInput")
    idx_in = nc.dram_tensor("idx", [P, P + T + 1], f32, kind="ExternalInput")
    iotab_in = nc.dram_tensor("iotab", [P, P], bf16, kind="ExternalInput")
    out_t = nc.dram_tensor("out", [P, D], f32, kind="ExternalOutput")

    n_slabs = T // SLAB
    assert n_slabs * SLAB == T

    with tile.TileContext(nc) as tc:
        with (
            tc.tile_pool(name="const", bufs=1) as constp,
            tc.tile_pool(name="scr", bufs=max(1, n_slabs)) as scrp,
            tc.tile_pool(name="scr2", bufs=max(1, n_slabs)) as scr2p,
            tc.tile_pool(name="slabs", bufs=8) as slabp,
            tc.tile_pool(name="ohp", bufs=OH_BUFS) as ohp,
            tc.tile_pool(name="psum", bufs=1, space="PSUM") as psump,
            tc.tile_pool(name="outp", bufs=1) as outp,
        ):
            idx_sb = constp.tile([P, P + T + 1], f32)
            nc.sync.dma_start(idx_sb[:], idx_in[:])
            iotab_sb = constp.tile([P, P], bf16)
            nc.sync.dma_start(iotab_sb[:], iotab_in[:])


            acc = psump.tile([P, 2 * D], f32)

            # Last matmul of each slab; the oh slot-reuse WAR hazard for
            # slab g is the last matmul of slab g-OH_BUFS. DVE compute
            # instructions only have one sync-wait slot, so a nop carrier
            # absorbs the cross-engine (PE) wait and the compare keeps
            # only its same-engine WAW wait.
            last_mm = {}

            for g in range(n_slabs):
                slab = slabp.tile([P, SLAB * 2 * D], bf16)
                if g >= 8:
                    scr2 = scr2p.tile([1, 2], f32, name="scr2")
                    dcar = nc.gpsimd.memset(scr2[:], 0.0)
                    add_dep_helper(
                        dcar.ins, last_mm[g - 8].ins, True, "slab WAR carrier"
                    )
                dma = nc.gpsimd.dma_start(
                    slab[:], h_in[:, g * SLAB * 2 * D : (g + 1) * SLAB * 2 * D]
                )
                if g >= 8:
                    add_dep_helper(dma.ins, dcar.ins, False, "dma after carrier")
                carrier = None
                if g >= OH_BUFS:
                    scr = scrp.tile([1, 8], f32, name=f"scr")
                    carrier = nc.vector.tensor_copy(out=scr[:], in_=idx_sb[0:1, 0:8])
                    add_dep_helper(
                        carrier.ins, last_mm[g - OH_BUFS].ins, True, "oh WAR carrier"
                    )
                oh_slab = ohp.tile([P, SLAB * P], bf16)
                for i in range(SLAB):
                    t = g * SLAB + i
                    ts = nc.vector.tensor_scalar(
                        out=oh_slab[:, i * P : (i + 1) * P],
                        in0=iotab_sb[:],
                        scalar1=idx_sb[:, P + t : P + t + 1],
                        scalar2=None,
                        op0=mybir.AluOpType.is_equal,
                    )
                    if carrier is not None and i == 0:
                        add_dep_helper(
                            ts.ins, carrier.ins, False, "compare after carrier"
                        )
                for i in range(SLAB):
                    t = g * SLAB + i
                    mm = nc.tensor.matmul(
                        out=acc[:],
                        lhsT=oh_slab[:, i * P : (i + 1) * P],
                        rhs=slab[:, i * 2 * D : (i + 1) * 2 * D],
                        start=(t == 0),
                        stop=(t == T - 1),
                    )
                last_mm[g] = mm

            hi_sb = outp.tile([P, D], f32)
            nc.vector.tensor_copy(out=hi_sb[:], in_=acc[:, 0:D])
            ssum = outp.tile([P, D], f32)
            nc.vector.tensor_tensor(
                out=ssum[:],
                in0=hi_sb[:],
                in1=acc[:, D : 2 * D],
                op=mybir.AluOpType.add,
            )
            res = outp.tile([P, D], f32)
            nc.vector.tensor_tensor(
                out=res[:],
                in0=ssum[:],
                in1=idx_sb[:, P + T : P + T + 1].to_broadcast([P, D]),
                op=mybir.AluOpType.mult,
            )
            nc.sync.dma_start(out_t[:], res[:])

    nc.finalize()
    return nc


def kernel(node_h, node_batch, num_graphs):
    global LAST_RESULT
    node_h = np.asarray(node_h)
    nb = np.asarray(node_batch)
    ng = int(num_graphs)

    N = node_h.shape[0]
    if (
        ng != G
        or node_h.ndim != 2
        or node_h.shape[1] != D
        or nb.shape != (N,)
        or N % P != 0
        or N // P < 2 * SLAB
        or np.any(nb[:-1] > nb[1:])
        or nb[0] < 0
        or nb[-1] >= G
    ):
        return _np_fallback(node_h, node_batch, num_graphs)

    node_h = np.ascontiguousarray(node_h, dtype=np.float32)
    nb = nb.astype(np.int64)

    n_tiles = N // P
    seg_per_core = G // N_CORES
    counts = np.bincount(nb, minlength=G)
    bounds = np.concatenate([[0], np.cumsum(counts)])
    starts = bounds[np.arange(N_CORES) * seg_per_core]
    ends = bounds[(np.arange(N_CORES) + 1) * seg_per_core]
    lo_t = starts // P
    hi_t = -(-ends // P)
    span = int((hi_t - lo_t).max())
    T = ((span + SLAB - 1) // SLAB) * SLAB
    if T > n_tiles:
        return _np_fallback(node_h, node_batch, num_graphs)
    lo = np.minimum(lo_t, n_tiles - T).astype(np.int64)

    iotab = np.ascontiguousarray(
        np.tile(np.arange(P, dtype=np.float32), (P, 1)).astype(BF16)
    )
    in_maps = []
    for c in range(N_CORES):
        r0 = int(lo[c]) * P
        r1 = r0 + T * P
        rows = node_h[r0:r1]
        hi = rows.astype(BF16)
        lo_res = (rows - hi.astype(np.float32)).astype(BF16)
        packed = np.empty((P, T, 2 * D), dtype=BF16)
        packed[:, :, :D] = hi.reshape(T, P, D).transpose(1, 0, 2)
        packed[:, :, D:] = lo_res.reshape(T, P, D).transpose(1, 0, 2)
        del hi, lo_res

        iota = np.tile(np.arange(P, dtype=np.float32), (P, 1))
        r = nb[r0:r1] - c * seg_per_core
        idxv = np.where((r >= 0) & (r < P), r.astype(np.float32), SENTINEL)
        recip = (
            1.0
            / np.maximum(
                counts[c * seg_per_core : (c + 1) * seg_per_core], 1.0
            ).astype(np.float32)
        ).astype(np.float32).reshape(P, 1)
        idx_T = np.ascontiguousarray(
            np.concatenate(
                [iota, idxv.reshape(T, P).T, recip], axis=1
            ).astype(np.float32)
        )

        in_maps.append(
            {
                "h": packed.reshape(P, T * 2 * D),
                "idx": idx_T,
                "iotab": iotab,
            }
        )

    if T not in _prog_cache:
        _prog_cache[T] = _build_program(T)
    nc = _prog_cache[T]

    from concourse.bass_utils import run_bass_kernel_spmd

    trace = bool(os.environ.get("KERNEL_TRACE"))
    result = run_bass_kernel_spmd(
        nc,
        in_maps,
        core_ids=list(range(N_CORES)),
        trace=trace,
        trace_cores=list(range(N_CORES)) if trace else None,
    )
    LAST_RESULT = result

    out = np.concatenate([result.results[c]["out"] for c in range(N_CORES)], axis=0)
    return out.astype(np.float32)


# revision 18
# speedup vs baseline: 1.0107x; 1.0107x over previous
"""Segment-mean (graph pooling) kernel for Trainium2, 8 NeuronCores.

reference semantics:
    sums   = segment_sum(node_h, node_batch, num_segments=G)
    counts = segment_sum(ones(N), node_batch, G)
    out    = sums / max(counts, 1)[:, None]

node_batch is sorted, so segments are contiguous row runs. Sharding:
core c owns segments [128c, 128(c+1)) and streams the node rows that
cover them. Per 128-row tile the DVE builds a one-hot selector
(iota == local_seg_id) and the PE accumulates onehot.T @ tile into a
single PSUM tile [128 segs, 256] (fp32 hi/lo bf16 split in the free
dim, exact to ~2^-18). Epilogue adds the halves and scales by
1/max(count,1).
"""

import os

import numpy as np
import ml_dtypes

BF16 = ml_dtypes.bfloat16
P = 128  # partitions / nodes per tile / segments per core
D = 128  # feature dim
G = 1024  # num segments
N_CORES = 8
SLAB = 32  # node-tiles per DMA slab (2 MiB per slab)
TT_CHUNK = 8  # node-tiles per fused DVE compare
SENTINEL = 200.0  # local seg id outside [0, 128) -> all-zero one-hot column

_prog_cache: dict[int, object] = {}
LAST_RESULT = None  # BassKernelResults of the most recent device run


def _np_fallback(node_h, node_batch, num_graphs):
    node_h = np.asarray(node_h, dtype=np.float32)
    nb = np.asarray(node_batch).astype(np.int64)
    ng = int(num_graphs)
    sums = np.zeros((ng, node_h.shape[1]), dtype=np.float32)
    np.add.at(sums, nb, node_h)
    counts = np.bincount(nb, minlength=ng).astype(np.float32)
    return sums / np.maximum(counts, 1.0)[:, None]


def _build_program(T: int):
    import concourse.bacc as bacc
    import concourse.mybir as mybir
    import concourse.tile as tile
    from concourse.tile import add_dep_helper

    OH_BUFS = 4

    bf16 = mybir.dt.bfloat16
    f32 = mybir.dt.float32

    nc = bacc.Bacc(None)
    h_in = nc.dram_tensor("h", [P, T * 2 * D], bf16, kind="ExternalInput")
    idx_in = nc.dram_tensor("idx", [P, P + T + 1], f32, kind="ExternalInput")
    iotab_in = nc.dram_tensor("iotab", [P, P], bf16, kind="ExternalInput")
    out_t = nc.dram_tensor("out", [P, D], f32, kind="ExternalOutput")

    n_slabs = T // SLAB
    assert n_slabs * SLAB == T

    with tile.TileContext(nc) as tc:
        with (
            tc.tile_pool(name="const", bufs=1) as constp,
            tc.tile_pool(name="scr", bufs=max(1, n_slabs)) as scrp,
            tc.tile_pool(name="scr2", bufs=max(1, n_slabs)) as scr2p,
            tc.tile_pool(name="slabs", bufs=4) as slabp,
            tc.tile_pool(name="ohp", bufs=OH_BUFS) as ohp,
            tc.tile_pool(name="psum", bufs=1, space="PSUM") as psump,
            tc.tile_pool(name="outp", bufs=1) as outp,
        ):
            idx_sb = constp.tile([P, P + T + 1], f32)
            head = P + 2 * SLAB
            nc.sync.dma_start(idx_sb[:, 0:head], idx_in[:, 0:head])
            nc.sync.dma_start(idx_sb[:, head:], idx_in[:, head:])
            iotab_sb = constp.tile([P, P], bf16)
            nc.sync.dma_start(iotab_sb[:], iotab_in[:])


            acc = psump.tile([P, 2 * D], f32)

            # Last matmul of each slab; the oh slot-reuse WAR hazard for
            # slab g is the last matmul of slab g-OH_BUFS. DVE compute
            # instructions only have one sync-wait slot, so a nop carrier
            # absorbs the cross-engine (PE) wait and the compare keeps
            # only its same-engine WAW wait.
            last_mm = {}

            for g in range(n_slabs):
                slab = slabp.tile([P, SLAB * 2 * D], bf16)
                if g >= 4:
                    scr2 = scr2p.tile([1, 2], f32, name="scr2")
                    dcar = nc.gpsimd.memset(scr2[:], 0.0)
                    add_dep_helper(
                        dcar.ins, last_mm[g - 4].ins, True, "slab WAR carrier"
                    )
                dma = nc.gpsimd.dma_start(
                    slab[:], h_in[:, g * SLAB * 2 * D : (g + 1) * SLAB * 2 * D]
                )
                if g >= 4:
                    add_dep_helper(dma.ins, dcar.ins, False, "dma after carrier")
                carrier = None
                if g >= OH_BUFS:
                    scr = scrp.tile([1, 8], f32, name=f"scr")
                    carrier = nc.vector.tensor_copy(out=scr[:], in_=idx_sb[0:1, 0:8])
                    add_dep_helper(
                        carrier.ins, last_mm[g - OH_BUFS].ins, True, "oh WAR carrier"
                    )
                oh_slab = ohp.tile([P, SLAB * P], bf16)
                iota_rep = (
                    idx_sb[:, 0:P].unsqueeze(1).to_broadcast([P, TT_CHUNK, P])
                )
                for q in range(SLAB // TT_CHUNK):
                    c0 = P + g * SLAB + q * TT_CHUNK
                    idx_rep = (
                        idx_sb[:, c0 : c0 + TT_CHUNK]
                        .unsqueeze(2)
                        .to_broadcast([P, TT_CHUNK, P])
                    )
                    tt = nc.vector.tensor_tensor(
                        out=oh_slab[
                            :, q * TT_CHUNK * P : (q + 1) * TT_CHUNK * P
                        ].rearrange("p (a b) -> p a b", b=P),
                        in0=iota_rep,
                        in1=idx_rep,
                        op=mybir.AluOpType.is_equal,
                    )
                    if carrier is not None and q == 0:
                        add_dep_helper(
                            tt.ins, carrier.ins, False, "compare after carrier"
                        )
                for i in range(SLAB):
                    t = g * SLAB + i
                    mm = nc.tensor.matmul(
                        out=acc[:],
                        lhsT=oh_slab[:, i * P : (i + 1) * P],
                        rhs=slab[:, i * 2 * D : (i + 1) * 2 * D],
                        start=(t == 0),
                        stop=(t == T - 1),
                    )
                last_mm[g] = mm

            hi_sb = outp.tile([P, D], f32)
            nc.vector.tensor_copy(out=hi_sb[:], in_=acc[:, 0:D])
            ssum = outp.tile([P, D], f32)
            nc.vector.tensor_tensor(
                out=ssum[:],
                in0=hi_sb[:],
                in1=acc[:, D : 2 * D],
                op=mybir.AluOpType.add,
            )
            res = outp.tile([P, D], f32)
            nc.vector.tensor_tensor(
                out=res[:],
                in0=ssum[:],
                in1=idx_sb[:, P + T : P + T + 1].to_broadcast([P, D]),
                op=mybir.AluOpType.mult,
            )
            nc.sync.dma_start(out_t[:], res[:])

    nc.finalize()
    return nc


def kernel(node_h, node_batch, num_graphs):
    global LAST_RESULT
    node_h = np.asarray(node_h)
    nb = np.asarray(node_batch)
    ng = int(num_graphs)

    N = node_h.shape[0]
    if (
        ng != G
        or node_h.ndim != 2
        or node_h.shape[1] != D
        or nb.shape != (N,)
        or N % P != 0
        or N // P < 2 * SLAB
        or np.any(nb[:-1] > nb[1:])
        or nb[0] < 0
        or nb[-1] >= G
    ):
        return _np_fallback(node_h, node_batch, num_graphs)

    node_h = np.ascontiguousarray(node_h, dtype=np.float32)
    nb = nb.astype(np.int64)

    n_tiles = N // P
    seg_per_core = G // N_CORES
    counts = np.bincount(nb, minlength=G)
    bounds = np.concatenate([[0], np.cumsum(counts)])
    starts = bounds[np.arange(N_CORES) * seg_per_core]
    ends = bounds[(np.arange(N_CORES) + 1) * seg_per_core]
    lo_t = starts // P
    hi_t = -(-ends // P)
    span = int((hi_t - lo_t).max())
    T = ((span + SLAB - 1) // SLAB) * SLAB
    if T > n_tiles:
        return _np_fallback(node_h, node_batch, num_graphs)
    lo = np.minimum(lo_t, n_tiles - T).astype(np.int64)

    iotab = np.ascontiguousarray(
        np.tile(np.arange(P, dtype=np.float32), (P, 1)).astype(BF16)
    )
    in_maps = []
    for c in range(N_CORES):
        r0 = int(lo[c]) * P
        r1 = r0 + T * P
        rows = node_h[r0:r1]
        hi = rows.astype(BF16)
        lo_res = (rows - hi.astype(np.float32)).astype(BF16)
        packed = np.empty((P, T, 2 * D), dtype=BF16)
        packed[:, :, :D] = hi.reshape(T, P, D).transpose(1, 0, 2)
        packed[:, :, D:] = lo_res.reshape(T, P, D).transpose(1, 0, 2)
        del hi, lo_res

        iota = np.tile(np.arange(P, dtype=np.float32), (P, 1))
        r = nb[r0:r1] - c * seg_per_core
        idxv = np.where((r >= 0) & (r < P), r.astype(np.float32), SENTINEL)
        recip = (
            1.0
            / np.maximum(
                counts[c * seg_per_core : (c + 1) * seg_per_core], 1.0
            ).astype(np.float32)
        ).astype(np.float32).reshape(P, 1)
        idx_T = np.ascontiguousarray(
            np.concatenate(
                [iota, idxv.reshape(T, P).T, recip], axis=1
            ).astype(np.float32)
        )

        in_maps.append(
            {
                "h": packed.reshape(P, T * 2 * D),
                "idx": idx_T,
                "iotab": iotab,
            }
        )

    if T not in _prog_cache:
        _prog_cache[T] = _build_program(T)
    nc = _prog_cache[T]

    from concourse.bass_utils import run_bass_kernel_spmd

    trace = bool(os.environ.get("KERNEL_TRACE"))
    result = run_bass_kernel_spmd(
        nc,
        in_maps,
        core_ids=list(range(N_CORES)),
        trace=trace,
        trace_cores=list(range(N_CORES)) if trace else None,
    )
    LAST_RESULT = result

    out = np.concatenate([result.results[c]["out"] for c in range(N_CORES)], axis=0)
    return out.astype(np.float32)


# revision 20
# speedup vs baseline: 1.0142x; 1.0035x over previous
"""Segment-mean (graph pooling) kernel for Trainium2, 8 NeuronCores.

reference semantics:
    sums   = segment_sum(node_h, node_batch, num_segments=G)
    counts = segment_sum(ones(N), node_batch, G)
    out    = sums / max(counts, 1)[:, None]

node_batch is sorted, so segments are contiguous row runs. Sharding:
core c owns segments [128c, 128(c+1)) and streams the node rows that
cover them. Per 128-row tile the DVE builds a one-hot selector
(iota == local_seg_id) and the PE accumulates onehot.T @ tile into a
single PSUM tile [128 segs, 256] (fp32 hi/lo bf16 split in the free
dim, exact to ~2^-18). Epilogue adds the halves and scales by
1/max(count,1).
"""

import os

import numpy as np
import ml_dtypes

BF16 = ml_dtypes.bfloat16
P = 128  # partitions / nodes per tile / segments per core
D = 128  # feature dim
G = 1024  # num segments
N_CORES = 8
SLAB = 16  # node-tiles per DMA slab (1 MiB per slab)
TT_CHUNK = 8  # node-tiles per fused DVE compare
SENTINEL = 200.0  # local seg id outside [0, 128) -> all-zero one-hot column

_prog_cache: dict[int, object] = {}
LAST_RESULT = None  # BassKernelResults of the most recent device run


def _np_fallback(node_h, node_batch, num_graphs):
    node_h = np.asarray(node_h, dtype=np.float32)
    nb = np.asarray(node_batch).astype(np.int64)
    ng = int(num_graphs)
    sums = np.zeros((ng, node_h.shape[1]), dtype=np.float32)
    np.add.at(sums, nb, node_h)
    counts = np.bincount(nb, minlength=ng).astype(np.float32)
    return sums / np.maximum(counts, 1.0)[:, None]


def _build_program(T: int):
    import concourse.bacc as bacc
    import concourse.mybir as mybir
    import concourse.tile as tile
    from concourse.tile import add_dep_helper

    OH_BUFS = 4

    bf16 = mybir.dt.bfloat16
    f32 = mybir.dt.float32

    nc = bacc.Bacc(None)
    h_in = nc.dram_tensor("h", [P, T * 2 * D], bf16, kind="ExternalInput")
    idx_in = nc.dram_tensor("idx", [P, P + T], bf16, kind="ExternalInput")
    recip_in = nc.dram_tensor("recip", [P, 1], f32, kind="ExternalInput")
    out_t = nc.dram_tensor("out", [P, D], f32, kind="ExternalOutput")

    n_slabs = T // SLAB
    assert n_slabs * SLAB == T

    with tile.TileContext(nc) as tc:
        with (
            tc.tile_pool(name="const", bufs=1) as constp,
            tc.tile_pool(name="scr", bufs=max(1, n_slabs)) as scrp,
            tc.tile_pool(name="scr2", bufs=max(1, n_slabs)) as scr2p,
            tc.tile_pool(name="slabs", bufs=10) as slabp,
            tc.tile_pool(name="ohp", bufs=OH_BUFS) as ohp,
            tc.tile_pool(name="psum", bufs=1, space="PSUM") as psump,
            tc.tile_pool(name="outp", bufs=1) as outp,
        ):
            idx_sb = constp.tile([P, P + T], bf16)
            head = P + 4 * SLAB
            nc.sync.dma_start(idx_sb[:, 0:head], idx_in[:, 0:head])
            nc.sync.dma_start(idx_sb[:, head:], idx_in[:, head:])
            recip_sb = constp.tile([P, 1], f32)
            nc.sync.dma_start(recip_sb[:], recip_in[:])


            acc = psump.tile([P, 2 * D], f32)

            # Last matmul of each slab; the oh slot-reuse WAR hazard for
            # slab g is the last matmul of slab g-OH_BUFS. DVE compute
            # instructions only have one sync-wait slot, so a nop carrier
            # absorbs the cross-engine (PE) wait and the compare keeps
            # only its same-engine WAW wait.
            last_mm = {}

            for g in range(n_slabs):
                slab = slabp.tile([P, SLAB * 2 * D], bf16)
                if g >= 10:
                    scr2 = scr2p.tile([1, 2], f32, name="scr2")
                    dcar = nc.gpsimd.memset(scr2[:], 0.0)
                    add_dep_helper(
                        dcar.ins, last_mm[g - 10].ins, True, "slab WAR carrier"
                    )
                dma = nc.gpsimd.dma_start(
                    slab[:], h_in[:, g * SLAB * 2 * D : (g + 1) * SLAB * 2 * D]
                )
                if g >= 10:
                    add_dep_helper(dma.ins, dcar.ins, False, "dma after carrier")
                carrier = None
                if g >= OH_BUFS:
                    scr = scrp.tile([1, 8], f32, name=f"scr")
                    carrier = nc.vector.tensor_copy(out=scr[:], in_=idx_sb[0:1, 0:8])
                    add_dep_helper(
                        carrier.ins, last_mm[g - OH_BUFS].ins, True, "oh WAR carrier"
                    )
                oh_slab = ohp.tile([P, SLAB * P], bf16)
                iota_rep = (
                    idx_sb[:, 0:P].unsqueeze(1).to_broadcast([P, TT_CHUNK, P])
                )
                for q in range(SLAB // TT_CHUNK):
                    c0 = P + g * SLAB + q * TT_CHUNK
                    idx_rep = (
                        idx_sb[:, c0 : c0 + TT_CHUNK]
                        .unsqueeze(2)
                        .to_broadcast([P, TT_CHUNK, P])
                    )
                    tt = nc.vector.tensor_tensor(
                        out=oh_slab[
                            :, q * TT_CHUNK * P : (q + 1) * TT_CHUNK * P
                        ].rearrange("p (a b) -> p a b", b=P),
                        in0=iota_rep,
                        in1=idx_rep,
                        op=mybir.AluOpType.is_equal,
                    )
                    if carrier is not None and q == 0:
                        add_dep_helper(
                            tt.ins, carrier.ins, False, "compare after carrier"
                        )
                for i in range(SLAB):
                    t = g * SLAB + i
                    mm = nc.tensor.matmul(
                        out=acc[:],
                        lhsT=oh_slab[:, i * P : (i + 1) * P],
                        rhs=slab[:, i * 2 * D : (i + 1) * 2 * D],
                        start=(t == 0),
                        stop=(t == T - 1),
                    )
                last_mm[g] = mm

            hi_sb = outp.tile([P, D], f32)
            nc.vector.tensor_copy(out=hi_sb[:], in_=acc[:, 0:D])
            ssum = outp.tile([P, D], f32)
            nc.vector.tensor_tensor(
                out=ssum[:],
                in0=hi_sb[:],
                in1=acc[:, D : 2 * D],
                op=mybir.AluOpType.add,
            )
            res = outp.tile([P, D], f32)
            nc.vector.tensor_tensor(
                out=res[:],
                in0=ssum[:],
                in1=recip_sb[:, 0:1].to_broadcast([P, D]),
                op=mybir.AluOpType.mult,
            )
            nc.sync.dma_start(out_t[:], res[:])

    nc.finalize()
    return nc


def kernel(node_h, node_batch, num_graphs):
    global LAST_RESULT
    node_h = np.asarray(node_h)
    nb = np.asarray(node_batch)
    ng = int(num_graphs)

    N = node_h.shape[0]
    if (
        ng != G
        or node_h.ndim != 2
        or node_h.shape[1] != D
        or nb.shape != (N,)
        or N % P != 0
        or N // P < 2 * SLAB
        or np.any(nb[:-1] > nb[1:])
        or nb[0] < 0
        or nb[-1] >= G
    ):
        return _np_fallback(node_h, node_batch, num_graphs)

    node_h = np.ascontiguousarray(node_h, dtype=np.float32)
    nb = nb.astype(np.int64)

    n_tiles = N // P
    seg_per_core = G // N_CORES
    counts = np.bincount(nb, minlength=G)
    bounds = np.concatenate([[0], np.cumsum(counts)])
    starts = bounds[np.arange(N_CORES) * seg_per_core]
    ends = bounds[(np.arange(N_CORES) + 1) * seg_per_core]
    lo_t = starts // P
    hi_t = -(-ends // P)
    span = int((hi_t - lo_t).max())
    T = ((span + SLAB - 1) // SLAB) * SLAB
    if T > n_tiles:
        return _np_fallback(node_h, node_batch, num_graphs)
    lo = np.minimum(lo_t, n_tiles - T).astype(np.int64)

    iotab = np.ascontiguousarray(
        np.tile(np.arange(P, dtype=np.float32), (P, 1)).astype(BF16)
    )
    in_maps = []
    for c in range(N_CORES):
        r0 = int(lo[c]) * P
        r1 = r0 + T * P
        rows = node_h[r0:r1]
        hi = rows.astype(BF16)
        lo_res = (rows - hi.astype(np.float32)).astype(BF16)
        packed = np.empty((P, T, 2 * D), dtype=BF16)
        packed[:, :, :D] = hi.reshape(T, P, D).transpose(1, 0, 2)
        packed[:, :, D:] = lo_res.reshape(T, P, D).transpose(1, 0, 2)
        del hi, lo_res

        iota = np.tile(np.arange(P, dtype=np.float32), (P, 1))
        r = nb[r0:r1] - c * seg_per_core
        idxv = np.where((r >= 0) & (r < P), r.astype(np.float32), SENTINEL)
        recip = (
            1.0
            / np.maximum(
                counts[c * seg_per_core : (c + 1) * seg_per_core], 1.0
            ).astype(np.float32)
        ).astype(np.float32).reshape(P, 1)
        idx_T = np.ascontiguousarray(
            np.concatenate([iota, idxv.reshape(T, P).T], axis=1).astype(BF16)
        )

        in_maps.append(
            {
                "h": packed.reshape(P, T * 2 * D),
                "idx": idx_T,
                "recip": recip,
            }
        )

    if T not in _prog_cache:
        _prog_cache[T] = _build_program(T)
    nc = _prog_cache[T]

    from concourse.bass_utils import run_bass_kernel_spmd

    trace = bool(os.environ.get("KERNEL_TRACE"))
    result = run_bass_kernel_spmd(
        nc,
        in_maps,
        core_ids=list(range(N_CORES)),
        trace=trace,
        trace_cores=list(range(N_CORES)) if trace else None,
    )
    LAST_RESULT = result

    out = np.concatenate([result.results[c]["out"] for c in range(N_CORES)], axis=0)
    return out.astype(np.float32)


# revision 21
# speedup vs baseline: 1.0171x; 1.0028x over previous
"""Segment-mean (graph pooling) kernel for Trainium2, 8 NeuronCores.

reference semantics:
    sums   = segment_sum(node_h, node_batch, num_segments=G)
    counts = segment_sum(ones(N), node_batch, G)
    out    = sums / max(counts, 1)[:, None]

node_batch is sorted, so segments are contiguous row runs. Sharding:
core c owns segments [128c, 128(c+1)) and streams the node rows that
cover them. Per 128-row tile the DVE builds a one-hot selector
(iota == local_seg_id) and the PE accumulates onehot.T @ tile into a
single PSUM tile [128 segs, 256] (fp32 hi/lo bf16 split in the free
dim, exact to ~2^-18). Epilogue adds the halves and scales by
1/max(count,1).
"""

import os

import numpy as np
import ml_dtypes

BF16 = ml_dtypes.bfloat16
P = 128  # partitions / nodes per tile / segments per core
D = 128  # feature dim
G = 1024  # num segments
N_CORES = 8
SLAB = 16  # node-tiles per DMA slab (1 MiB per slab)
TT_CHUNK = 8  # node-tiles per fused DVE compare
SENTINEL = 200.0  # local seg id outside [0, 128) -> all-zero one-hot column

_prog_cache: dict[int, object] = {}
LAST_RESULT = None  # BassKernelResults of the most recent device run


def _np_fallback(node_h, node_batch, num_graphs):
    node_h = np.asarray(node_h, dtype=np.float32)
    nb = np.asarray(node_batch).astype(np.int64)
    ng = int(num_graphs)
    sums = np.zeros((ng, node_h.shape[1]), dtype=np.float32)
    np.add.at(sums, nb, node_h)
    counts = np.bincount(nb, minlength=ng).astype(np.float32)
    return sums / np.maximum(counts, 1.0)[:, None]


def _build_program(T: int):
    import concourse.bacc as bacc
    import concourse.mybir as mybir
    import concourse.tile as tile
    from concourse.tile import add_dep_helper

    OH_BUFS = 6

    bf16 = mybir.dt.bfloat16
    f32 = mybir.dt.float32

    nc = bacc.Bacc(None)
    h_in = nc.dram_tensor("h", [P, T * 2 * D], bf16, kind="ExternalInput")
    idx_in = nc.dram_tensor("idx", [P, P + T], bf16, kind="ExternalInput")
    recip_in = nc.dram_tensor("recip", [P, 1], f32, kind="ExternalInput")
    out_t = nc.dram_tensor("out", [P, D], f32, kind="ExternalOutput")

    n_slabs = T // SLAB
    assert n_slabs * SLAB == T

    with tile.TileContext(nc) as tc:
        with (
            tc.tile_pool(name="const", bufs=1) as constp,
            tc.tile_pool(name="scr", bufs=max(1, n_slabs)) as scrp,
            tc.tile_pool(name="scr2", bufs=max(1, n_slabs)) as scr2p,
            tc.tile_pool(name="slabs", bufs=12) as slabp,
            tc.tile_pool(name="ohp", bufs=OH_BUFS) as ohp,
            tc.tile_pool(name="psum", bufs=1, space="PSUM") as psump,
            tc.tile_pool(name="outp", bufs=1) as outp,
        ):
            idx_sb = constp.tile([P, P + T], bf16)
            head = P + 4 * SLAB
            nc.sync.dma_start(idx_sb[:, 0:head], idx_in[:, 0:head])
            nc.sync.dma_start(idx_sb[:, head:], idx_in[:, head:])
            recip_sb = constp.tile([P, 1], f32)
            nc.sync.dma_start(recip_sb[:], recip_in[:])


            acc = psump.tile([P, 2 * D], f32)

            # Last matmul of each slab; the oh slot-reuse WAR hazard for
            # slab g is the last matmul of slab g-OH_BUFS. DVE compute
            # instructions only have one sync-wait slot, so a nop carrier
            # absorbs the cross-engine (PE) wait and the compare keeps
            # only its same-engine WAW wait.
            last_mm = {}

            for g in range(n_slabs):
                slab = slabp.tile([P, SLAB * 2 * D], bf16)
                if g >= 12:
                    scr2 = scr2p.tile([1, 2], f32, name="scr2")
                    dcar = nc.gpsimd.memset(scr2[:], 0.0)
                    add_dep_helper(
                        dcar.ins, last_mm[g - 12].ins, True, "slab WAR carrier"
                    )
                dma = nc.gpsimd.dma_start(
                    slab[:], h_in[:, g * SLAB * 2 * D : (g + 1) * SLAB * 2 * D]
                )
                if g >= 12:
                    add_dep_helper(dma.ins, dcar.ins, False, "dma after carrier")
                carrier = None
                if g >= OH_BUFS:
                    scr = scrp.tile([1, 8], f32, name=f"scr")
                    carrier = nc.vector.tensor_copy(out=scr[:], in_=idx_sb[0:1, 0:8])
                    add_dep_helper(
                        carrier.ins, last_mm[g - OH_BUFS].ins, True, "oh WAR carrier"
                    )
                oh_slab = ohp.tile([P, SLAB * P], bf16)
                iota_rep = (
                    idx_sb[:, 0:P].unsqueeze(1).to_broadcast([P, TT_CHUNK, P])
                )
                for q in range(SLAB // TT_CHUNK):
                    c0 = P + g * SLAB + q * TT_CHUNK
                    idx_rep = (
                        idx_sb[:, c0 : c0 + TT_CHUNK]
                        .unsqueeze(2)
                        .to_broadcast([P, TT_CHUNK, P])
                    )
                    tt = nc.vector.tensor_tensor(
                        out=oh_slab[
                            :, q * TT_CHUNK * P : (q + 1) * TT_CHUNK * P
                        ].rearrange("p (a b) -> p a b", b=P),
                        in0=iota_rep,
                        in1=idx_rep,
                        op=mybir.AluOpType.is_equal,
                    )
                    if carrier is not None and q == 0:
                        add_dep_helper(
                            tt.ins, carrier.ins, False, "compare after carrier"
                        )
                for i in range(SLAB):
                    t = g * SLAB + i
                    mm = nc.tensor.matmul(
                        out=acc[:],
                        lhsT=oh_slab[:, i * P : (i + 1) * P],
                        rhs=slab[:, i * 2 * D : (i + 1) * 2 * D],
                        start=(t == 0),
                        stop=(t == T - 1),
                    )
                last_mm[g] = mm

            hi_sb = outp.tile([P, D], f32)
            nc.vector.tensor_copy(out=hi_sb[:], in_=acc[:, 0:D])
            ssum = outp.tile([P, D], f32)
            nc.vector.tensor_tensor(
                out=ssum[:],
                in0=hi_sb[:],
                in1=acc[:, D : 2 * D],
                op=mybir.AluOpType.add,
            )
            res = outp.tile([P, D], f32)
            nc.vector.tensor_tensor(
                out=res[:],
                in0=ssum[:],
                in1=recip_sb[:, 0:1].to_broadcast([P, D]),
                op=mybir.AluOpType.mult,
            )
            nc.sync.dma_start(out_t[:], res[:])

    nc.finalize()
    return nc


def kernel(node_h, node_batch, num_graphs):
    global LAST_RESULT
    node_h = np.asarray(node_h)
    nb = np.asarray(node_batch)
    ng = int(num_graphs)

    N = node_h.shape[0]
    if (
        ng != G
        or node_h.ndim != 2
        or node_h.shape[1] != D
        or nb.shape != (N,)
        or N % P != 0
        or N // P < 2 * SLAB
        or np.any(nb[:-1] > nb[1:])
        or nb[0] < 0
        or nb[-1] >= G
    ):
        return _np_fallback(node_h, node_batch, num_graphs)

    node_h = np.ascontiguousarray(node_h, dtype=np.float32)
    nb = nb.astype(np.int64)

    n_tiles = N // P
    seg_per_core = G // N_CORES
    counts = np.bincount(nb, minlength=G)
    bounds = np.concatenate([[0], np.cumsum(counts)])
    starts = bounds[np.arange(N_CORES) * seg_per_core]
    ends = bounds[(np.arange(N_CORES) + 1) * seg_per_core]
    lo_t = starts // P
    hi_t = -(-ends // P)
    span = int((hi_t - lo_t).max())
    T = ((span + SLAB - 1) // SLAB) * SLAB
    if T > n_tiles:
        return _np_fallback(node_h, node_batch, num_graphs)
    lo = np.minimum(lo_t, n_tiles - T).astype(np.int64)

    iotab = np.ascontiguousarray(
        np.tile(np.arange(P, dtype=np.float32), (P, 1)).astype(BF16)
    )
    in_maps = []
    for c in range(N_CORES):
        r0 = int(lo[c]) * P
        r1 = r0 + T * P
        rows = node_h[r0:r1]
        hi = rows.astype(BF16)
        lo_res = (rows - hi.astype(np.float32)).astype(BF16)
        packed = np.empty((P, T, 2 * D), dtype=BF16)
        packed[:, :, :D] = hi.reshape(T, P, D).transpose(1, 0, 2)
        packed[:, :, D:] = lo_res.reshape(T, P, D).transpose(1, 0, 2)
        del hi, lo_res

        iota = np.tile(np.arange(P, dtype=np.float32), (P, 1))
        r = nb[r0:r1] - c * seg_per_core
        idxv = np.where((r >= 0) & (r < P), r.astype(np.float32), SENTINEL)
        recip = (
            1.0
            / np.maximum(
                counts[c * seg_per_core : (c + 1) * seg_per_core], 1.0
            ).astype(np.float32)
        ).astype(np.float32).reshape(P, 1)
        idx_T = np.ascontiguousarray(
            np.concatenate([iota, idxv.reshape(T, P).T], axis=1).astype(BF16)
        )

        in_maps.append(
            {
                "h": packed.reshape(P, T * 2 * D),
                "idx": idx_T,
                "recip": recip,
            }
        )

    if T not in _prog_cache:
        _prog_cache[T] = _build_program(T)
    nc = _prog_cache[T]

    from concourse.bass_utils import run_bass_kernel_spmd

    trace = bool(os.environ.get("KERNEL_TRACE"))
    result = run_bass_kernel_spmd(
        nc,
        in_maps,
        core_ids=list(range(N_CORES)),
        trace=trace,
        trace_cores=list(range(N_CORES)) if trace else None,
    )
    LAST_RESULT = result

    out = np.concatenate([result.results[c]["out"] for c in range(N_CORES)], axis=0)
    return out.astype(np.float32)
